# revision 22
# baseline (speedup 1.0000x reference)
"""GQA kernel for 8 Trainium2 NeuronCores.

Problem: nn_GroupQueryAttention — B=2, S=2048, HIDDEN=2048, 32 heads,
8 kv-groups, head_dim 64.

Sharding: data parallel on batch (2) x tensor parallel on kv-groups (4
group-pairs). Core c owns batch c//4 and kv-groups {2*(c%4), 2*(c%4)+1}
(512 q-features, 128 kv-features). Each core computes a partial
out-projection (Wo columns of its features); host sums 4 partials per
batch.

Key optimizations over the fp32r baseline:
  - all matmul operands in bf16 (fp32r ran under a HW power throttle at
    ~1.2GHz; bf16 streams 1 col/cycle at full clock). PSUM accumulation
    stays fp32.
  - key compaction: the mask is per-key (broadcast over queries+heads),
    so masked keys contribute nothing anywhere. Host gathers the
    unmasked key positions, pads to a multiple of 128, and the kernel
    only projects/attends over the kept keys (~half of 2048 for the
    random mask). Padded key rows get an exp bias of -30000 so E=0.
  - heads sharing a (group, q-parity) pair up into 2*SQB-wide attention
    streams: one exp + one normalize chain per pair (PSUM matmul writes
    stay bank-scoped at SQB cols).
  - k/v projections run first behind per-ht-split weight DMAs so the PE
    starts ~2us in; wq/wo stream during kv-proj.
  - out-projection of q-tile i is interleaved into the attention pairs
    of q-tile i+1 to fill normalize-chain bubbles.
  - reciprocal_approx_fast for the softmax denominators; bf16 output
    partials summed on host in fp32.
"""

import numpy as np

B = 2
S = 2048
H = 2048
G = 8            # kv groups total
HPG = 4          # heads per group
D = 64           # head dim
NCORES = 8
QF = 512         # q features per core (2 groups * 4 heads * 64)
KF = 128         # kv features per core (2 groups * 64)
SCALE = 1.0 / np.sqrt(np.float32(D))
P = 128
SQA = 512        # seq chunk for projection phase (moving dim)
SQB = 512        # q tile for attention / out-proj phase
NHT = H // P     # 16 hidden partition tiles
NMT = QF // P    # 4 q-feature partition tiles
NQT = S // SQB   # 4 q tiles
MASK_NEG = -30000.0


def _build_bass(KB):
    """Build the per-core program for KPAD = KB*128 kept+padded keys."""
    from contextlib import ExitStack

    import concourse.tile as tile
    from concourse import bacc, mybir

    f32 = mybir.dt.float32
    bf16 = mybir.dt.bfloat16
    Exp = mybir.ActivationFunctionType.Exp
    KPAD = KB * P

    nc = bacc.Bacc("TRN2", target_bir_lowering=False, debug=False)

    xT = nc.dram_tensor("xT", [H, S], bf16, kind="ExternalInput").ap()
    xkT = nc.dram_tensor("xkT", [H, KPAD], bf16, kind="ExternalInput").ap()
    wqT = nc.dram_tensor("wqT", [H, QF], bf16, kind="ExternalInput").ap()
    wkT = nc.dram_tensor("wkT", [H, KF], bf16, kind="ExternalInput").ap()
    wvT = nc.dram_tensor("wvT", [H, KF], bf16, kind="ExternalInput").ap()
    woT = nc.dram_tensor("woT", [QF, H], bf16, kind="ExternalInput").ap()
    mb = nc.dram_tensor("mb", [P, KB], f32, kind="ExternalInput").ap()
    outT = nc.dram_tensor("outT", [H, S], bf16, kind="ExternalOutput").ap()

    xT_r = xT.rearrange("(t p) s -> p t s", p=P)
    xkT_r = xkT.rearrange("(t p) s -> p t s", p=P)
    wqT_r = wqT.rearrange("(t p) f -> p t f", p=P)
    wkT_r = wkT.rearrange("(t p) f -> p t f", p=P)
    wvT_r = wvT.rearrange("(t p) f -> p t f", p=P)
    woT_r = woT.rearrange("(t p) f -> p t f", p=P)

    with tile.TileContext(nc) as tc, ExitStack() as es:
        ec = es.enter_context
        ec(nc.allow_low_precision(reason="bf16 matmuls, fp32 PSUM accum"))
        const_pool = ec(tc.tile_pool(name="const", bufs=1))
        wq_pool = ec(tc.tile_pool(name="wq", bufs=1))
        wo_pool = ec(tc.tile_pool(name="wo", bufs=1))
        wkv_pool = ec(tc.tile_pool(name="wkv", bufs=1))
        xt_pool = ec(tc.tile_pool(name="xt", bufs=2))
        xk_pool = ec(tc.tile_pool(name="xk", bufs=3))
        qt_pool = ec(tc.tile_pool(name="qt", bufs=1))
        kt_pool = ec(tc.tile_pool(name="kt", bufs=1))
        v_pool = ec(tc.tile_pool(name="vs", bufs=1))
        vt_pool = ec(tc.tile_pool(name="vt", bufs=2))
        at_pool = ec(tc.tile_pool(name="at", bufs=2))
        e_pool = ec(tc.tile_pool(name="e", bufs=4))
        rc_pool = ec(tc.tile_pool(name="rc", bufs=2))
        rb_pool = ec(tc.tile_pool(name="rb", bufs=2))
        out_pool = ec(tc.tile_pool(name="outs", bufs=3))
        psa_pool = ec(tc.tile_pool(name="psa", bufs=2, space="PSUM"))
        po_pool = ec(tc.tile_pool(name="po", bufs=2, space="PSUM"))
        pp_pool = ec(tc.tile_pool(name="pp", bufs=2, space="PSUM"))

        # ---- constants ----
        mb_sb = const_pool.tile([P, KB], f32, tag="mb")
        nc.sync.dma_start(out=mb_sb, in_=mb)
        ones_sb = const_pool.tile([1, D], bf16, tag="ones")
        nc.vector.memset(ones_sb, 1.0)

        # ---- k/v weights + gathered-key x, split per-ht so the PE can
        # start on ht=0 almost immediately; wq interleaves per-mt between
        # xk chunks (xk bufs=3 avoids DMA head-of-line blocking) ----
        wk_sb = wkv_pool.tile([P, NHT, KF], bf16, tag="wk")
        wv_sb = wkv_pool.tile([P, NHT, KF], bf16, tag="wv")
        nc.sync.dma_start(out=wk_sb, in_=wkT_r)
        nc.sync.dma_start(out=wv_sb, in_=wvT_r)

        wq_sb = wq_pool.tile([P, NHT, QF], bf16, tag="wq")
        nkc = (KPAD + SQA - 1) // SQA
        xk_tiles = []
        for c in range(nkc):
            k0 = c * SQA
            kw = min(SQA, KPAD - k0)
            xk = xk_pool.tile([P, NHT, SQA], bf16, tag="xk", name=f"xk{c}")
            if c == 0:
                # split the first chunk so the PE starts after ~1/4 of it
                for hq in range(4):
                    nc.sync.dma_start(
                        out=xk[:, 4 * hq:4 * hq + 4, 0:kw],
                        in_=xkT_r[:, 4 * hq:4 * hq + 4, k0:k0 + kw],
                    )
            else:
                nc.sync.dma_start(out=xk[:, :, 0:kw], in_=xkT_r[:, :, k0:k0 + kw])
            xk_tiles.append(xk)
        # wq in halves: q-proj half 0 (mts 0,1) can start sooner
        nc.sync.dma_start(out=wq_sb[:, :, 0:2 * P], in_=wqT_r[:, :, 0:2 * P])

        # kT stored twice: kta = [g0; g1] on partitions [0:64; 64:128],
        # ktb = [g1; g0] — so any (group, q-parity) pair can be read at
        # the base partition (matmul requires lhsT base == rhs base).
        kta_sb = kt_pool.tile([P, KPAD], bf16, tag="kta")
        ktb_sb = kt_pool.tile([P, KPAD], bf16, tag="ktb")
        # per-group v tiles: [v (64) | ones] per key block (XBAR transpose
        # output must land at an aligned free offset, so each group gets
        # its own tile with the transpose writing at offset 0)
        # kb-block stride padded to 128 so every transpose destination
        # offset is XBAR-aligned; cols 65..127 are unused padding
        v_g = [
            v_pool.tile([P, KB, P], bf16, tag=f"v{g}", name=f"v{g}")
            for g in range(2)
        ]
        onescol_sb = const_pool.tile([P, KB], bf16, tag="onescol")
        nc.vector.memset(onescol_sb, 1.0)
        nc.vector.tensor_copy(v_g[0][:, :, 64], onescol_sb)
        nc.vector.tensor_copy(v_g[1][:, :, 64], onescol_sb)

        # ---- phase A1: k/v projections over kept keys ----
        for c in range(nkc):
            k0 = c * SQA
            kw = min(SQA, KPAD - k0)
            xk = xk_tiles[c]
            ps = pp_pool.tile([P, SQA], f32, tag="pp", name="ps_k")
            for ht in range(NHT):
                nc.tensor.matmul(
                    ps[:, 0:kw],
                    lhsT=wk_sb[:, ht, :],
                    rhs=xk[:, ht, 0:kw],
                    start=(ht == 0),
                    stop=(ht == NHT - 1),
                )
            nc.scalar.copy(kta_sb[:, k0:k0 + kw], ps[:, 0:kw])
            nc.vector.tensor_copy(ktb_sb[0:64, k0:k0 + kw], ps[64:128, 0:kw])
            nc.vector.tensor_copy(ktb_sb[64:128, k0:k0 + kw], ps[0:64, 0:kw])
            # v as vT (wide streams), then XBAR DMA-transpose per key
            # block into the key-major layout (zero PE cost)
            psv = pp_pool.tile([P, SQA], f32, tag="pp", name="ps_v")
            for ht in range(NHT):
                nc.tensor.matmul(
                    psv[:, 0:kw],
                    lhsT=wv_sb[:, ht, :],
                    rhs=xk[:, ht, 0:kw],
                    start=(ht == 0),
                    stop=(ht == NHT - 1),
                )
            # per-group vT halves at partition base 0 (XBAR transpose
            # misreads nonzero partition bases)
            vt_g0 = vt_pool.tile([D, SQA], bf16, tag="vt0", name=f"vt0_{c}")
            vt_g1 = vt_pool.tile([D, SQA], bf16, tag="vt1", name=f"vt1_{c}")
            nc.scalar.copy(vt_g0[:, 0:kw], psv[0:64, 0:kw])
            nc.scalar.copy(vt_g1[:, 0:kw], psv[64:128, 0:kw])
            for st in range(kw // P):
                kb = (k0 + st * P) // P
                nc.scalar.dma_start_transpose(
                    out=v_g[0][:, kb, 0:64],
                    in_=vt_g0[:, st * P:(st + 1) * P],
                )
                nc.scalar.dma_start_transpose(
                    out=v_g[1][:, kb, 0:64],
                    in_=vt_g1[:, st * P:(st + 1) * P],
                )

        # wo streams in after xt0 (overlaps q-proj + attention start)
        wo_sb = wo_pool.tile([P, NMT, H], bf16, tag="wo")

        # ---- q projection, one q-tile at a time (xt DMA + wo after
        # tile 0's, interleaved into attention of the previous tile) ----
        # qT stored with col = (qtile, mt, q) so a head-pair's q tile is
        # one contiguous 2*SQB range (matmul moving AP must be 1D).
        qt_sb = qt_pool.tile([P, S * NMT], bf16, tag="qt")
        xt_tiles = {}

        def xt_dma(qt):
            s0 = qt * SQB
            xt = xt_pool.tile([P, NHT, SQB], bf16, tag="xt", name=f"xt{qt}")
            nc.sync.dma_start(out=xt, in_=xT_r[:, :, s0:s0 + SQB])
            xt_tiles[qt] = xt

        def qproj_half(qt, half):
            xt = xt_tiles[qt]
            for mt in range(2 * half, 2 * half + 2):
                ps = pp_pool.tile([P, SQB], f32, tag="pp", name="ps_q")
                for ht in range(NHT):
                    nc.tensor.matmul(
                        ps,
                        lhsT=wq_sb[:, ht, mt * P:(mt + 1) * P],
                        rhs=xt[:, ht, :],
                        start=(ht == 0),
                        stop=(ht == NHT - 1),
                    )
                c0 = (qt * NMT + mt) * SQB
                nc.scalar.copy(qt_sb[:, c0:c0 + SQB], ps)

        xt_dma(0)
        nc.sync.dma_start(
            out=wq_sb[:, :, 2 * P:4 * P], in_=wqT_r[:, :, 2 * P:4 * P]
        )
        nc.sync.dma_start(out=wo_sb, in_=woT_r)
        xt_dma(1)
        qproj_half(0, 0)

        # ---- phase B/C: attention per q tile; q-proj of tile qt+1 and
        # out-projection of tile qt-1 fill the pair boundaries ----
        # Heads sharing a (group, q-parity) merge into one 2*SQB-wide
        # stream: pair hp covers heads at mt0=2*(hp//2)+{0,1}, partition
        # base r0=64*(hp%2), group g=hp//2.
        at_tiles = {}

        def outproj_8(qt, part):
            q0 = qt * SQB
            for mt in range(part * 8, part * 8 + 8):
                ps = pp_pool.tile([P, SQB], f32, tag="pp", name="ps_o")
                for kb4 in range(NMT):
                    nc.tensor.matmul(
                        ps,
                        lhsT=wo_sb[:, kb4, mt * P:(mt + 1) * P],
                        rhs=at_tiles[(qt, kb4)][:, :],
                        start=(kb4 == 0),
                        stop=(kb4 == NMT - 1),
                    )
                ot = out_pool.tile([P, SQB], bf16, tag="ot")
                nc.vector.tensor_copy(ot, ps)
                nc.gpsimd.dma_start(
                    out=outT[mt * P:(mt + 1) * P, q0:q0 + SQB], in_=ot
                )

        for qt in range(NQT):
            for mt in range(NMT):
                at_tiles[(qt, mt)] = at_pool.tile(
                    [P, SQB], bf16, tag=f"at{mt}", name=f"at_{qt}_{mt}"
                )
            for hp in range(4):
                g = hp // 2
                par = hp % 2
                r0 = par * D
                mt0 = 2 * g
                kt_src = kta_sb if g == par else ktb_sb
                po = [
                    po_pool.tile([65, SQB], f32, tag="po", name=f"po{j}")
                    for j in range(2)
                ]
                for kb in range(KB):
                    # matmul PSUM writes are bank-scoped (<=512 f32
                    # cols): two matmuls fill the wide tile's halves.
                    ps = psa_pool.tile([P, 2 * SQB], f32, tag="ps")
                    for j in range(2):
                        nc.tensor.matmul(
                            ps[:, j * SQB:(j + 1) * SQB],
                            lhsT=kt_src[r0:r0 + D, kb * P:(kb + 1) * P],
                            rhs=qt_sb[r0:r0 + D,
                                      (qt * NMT + mt0 + j) * SQB:
                                      (qt * NMT + mt0 + j + 1) * SQB],
                            start=True,
                            stop=True,
                        )
                    e = e_pool.tile([P, 2 * SQB], bf16, tag="e")
                    nc.scalar.activation(
                        e, ps, Exp,
                        bias=mb_sb[:, kb:kb + 1], scale=float(SCALE),
                    )
                    for j in range(2):
                        nc.tensor.matmul(
                            po[j],
                            lhsT=v_g[g][:, kb, 0:65],
                            rhs=e[:, j * SQB:(j + 1) * SQB],
                            start=(kb == 0),
                            stop=(kb == KB - 1),
                        )
                # normalize: rows 0..63 are numerator^T, row 64 denominator
                # (copy denom to partition 0 first: custom-DVE recip
                # mis-reads nonzero partition bases)
                dn = rc_pool.tile([1, 2 * SQB], f32, tag="dn")
                for j in range(2):
                    nc.scalar.copy(
                        dn[:, j * SQB:(j + 1) * SQB], po[j][64:65, :]
                    )
                rc = rc_pool.tile([1, 2 * SQB], f32, tag="rc")
                nc.vector.reciprocal_approx_fast(rc, dn)
                rb = rb_pool.tile([D, 2 * SQB], f32, tag="rb")
                nc.gpsimd.partition_broadcast(rb, rc)
                for j in range(2):
                    nc.vector.tensor_mul(
                        at_tiles[(qt, mt0 + j)][r0:r0 + D, :],
                        po[j][0:64, :],
                        rb[:, j * SQB:(j + 1) * SQB],
                    )
                # fill the pair boundary with independent PE work
                if qt == 0:
                    if hp == 0:
                        qproj_half(0, 1)
                    elif hp == 1:
                        qproj_half(1, 0)
                    elif hp == 2:
                        qproj_half(1, 1)
                    else:
                        xt_dma(2)
                elif qt < NQT - 1:
                    if hp <= 1:
                        qproj_half(qt + 1, hp)
                    else:
                        if hp == 2 and qt + 2 < NQT:
                            xt_dma(qt + 2)
                        outproj_8(qt - 1, hp - 2)
                else:
                    if hp >= 2:
                        outproj_8(qt - 1, hp - 2)
        for part in range(2):
            outproj_8(NQT - 1, part)
    nc.compile()
    return nc


_NC_CACHE = {}


def _get_nc(KB):
    if KB not in _NC_CACHE:
        _NC_CACHE[KB] = _build_bass(KB)
    return _NC_CACHE[KB]


def _make_in_maps(inputs):
    import ml_dtypes

    bf = ml_dtypes.bfloat16
    x = np.asarray(inputs["x"], dtype=np.float32)
    mask = np.asarray(inputs["mask"])
    Wq = np.asarray(inputs["Wq"], dtype=np.float32)
    Wk = np.asarray(inputs["Wk"], dtype=np.float32)
    Wv = np.asarray(inputs["Wv"], dtype=np.float32)
    Wo = np.asarray(inputs["Wo"], dtype=np.float32)

    # gather kept (unmasked) key positions per batch; pad to common KPAD
    idxs = [np.nonzero(mask[b, 0, 0, 0, :] != 0)[0] for b in range(B)]
    kept_max = max(1, max(len(i) for i in idxs))
    KB = (kept_max + P - 1) // P
    KPAD = KB * P

    xTs, xkTs, mbs = [], [], []
    for b in range(B):
        xb = x[b].astype(bf)
        xTs.append(np.ascontiguousarray(xb.T))
        xk = np.zeros((KPAD, H), dtype=bf)
        xk[: len(idxs[b])] = xb[idxs[b]]
        xkTs.append(np.ascontiguousarray(xk.T))
        bias = np.full(KPAD, np.float32(MASK_NEG), dtype=np.float32)
        bias[: len(idxs[b])] = 0.0
        mbs.append(np.ascontiguousarray(bias.reshape(KB, P).T))

    in_maps = []
    for c in range(NCORES):
        b, gp = divmod(c, 4)
        qs = slice(gp * QF, (gp + 1) * QF)
        ks = slice(gp * KF, (gp + 1) * KF)
        in_maps.append({
            "xT": xTs[b],
            "xkT": xkTs[b],
            "wqT": np.ascontiguousarray(Wq[qs, :].T.astype(bf)),
            "wkT": np.ascontiguousarray(Wk[ks, :].T.astype(bf)),
            "wvT": np.ascontiguousarray(Wv[ks, :].T.astype(bf)),
            "woT": np.ascontiguousarray(Wo[:, qs].T.astype(bf)),
            "mb": mbs[b],
        })
    return in_maps, KB


def kernel(**inputs):
    from concourse.bass_utils import run_bass_kernel_spmd

    in_maps, KB = _make_in_maps(inputs)
    nc = _get_nc(KB)
    res = run_bass_kernel_spmd(nc, in_maps, core_ids=list(range(NCORES)))
    outs = [np.asarray(r["outT"], dtype=np.float32) for r in res.results]
    out = np.empty((B, S, H), dtype=np.float32)
    for b in range(B):
        acc = outs[4 * b] + outs[4 * b + 1] + outs[4 * b + 2] + outs[4 * b + 3]
        out[b] = acc.T
    return out


# revision 24
# speedup vs baseline: 1.0988x; 1.0988x over previous
"""GQA kernel for 8 Trainium2 NeuronCores.

Problem: nn_GroupQueryAttention — B=2, S=2048, HIDDEN=2048, 32 heads,
8 kv-groups, head_dim 64.

Sharding: data parallel on batch (2) x tensor parallel on kv-groups (4
group-pairs). Core c owns batch c//4 and kv-groups {2*(c%4), 2*(c%4)+1}
(512 q-features, 128 kv-features). Each core computes a partial
out-projection (Wo columns of its features); host sums 4 partials per
batch.

Key optimizations over the fp32r baseline:
  - all matmul operands in bf16 (fp32r ran under a HW power throttle at
    ~1.2GHz; bf16 streams 1 col/cycle at full clock). PSUM accumulation
    stays fp32.
  - key compaction: the mask is per-key (broadcast over queries+heads),
    so masked keys contribute nothing anywhere. Host gathers the
    unmasked key positions, pads to a multiple of 128, and the kernel
    only projects/attends over the kept keys (~half of 2048 for the
    random mask). Padded key rows get an exp bias of -30000 so E=0.
  - heads sharing a (group, q-parity) pair up into 2*SQB-wide attention
    streams: one exp + one normalize chain per pair (PSUM matmul writes
    stay bank-scoped at SQB cols).
  - k/v projections run first behind per-ht-split weight DMAs so the PE
    starts ~2us in; wq/wo stream during kv-proj.
  - out-projection of q-tile i is interleaved into the attention pairs
    of q-tile i+1 to fill normalize-chain bubbles.
  - reciprocal_approx_fast for the softmax denominators; bf16 output
    partials summed on host in fp32.
"""

import numpy as np

B = 2
S = 2048
H = 2048
G = 8            # kv groups total
HPG = 4          # heads per group
D = 64           # head dim
NCORES = 8
QF = 512         # q features per core (2 groups * 4 heads * 64)
KF = 128         # kv features per core (2 groups * 64)
SCALE = 1.0 / np.sqrt(np.float32(D))
P = 128
SQA = 512        # seq chunk for projection phase (moving dim)
SQB = 512        # q tile for attention / out-proj phase
NHT = H // P     # 16 hidden partition tiles
NMT = QF // P    # 4 q-feature partition tiles
NQT = S // SQB   # 4 q tiles
MASK_NEG = -30000.0


def _build_bass(KB):
    """Build the per-core program for KPAD = KB*128 kept+padded keys."""
    from contextlib import ExitStack

    import concourse.tile as tile
    from concourse import bacc, mybir

    f32 = mybir.dt.float32
    bf16 = mybir.dt.bfloat16
    Exp = mybir.ActivationFunctionType.Exp
    KPAD = KB * P

    nc = bacc.Bacc("TRN2", target_bir_lowering=False, debug=False)

    xT = nc.dram_tensor("xT", [H, S], bf16, kind="ExternalInput").ap()
    xkT = nc.dram_tensor("xkT", [H, KPAD], bf16, kind="ExternalInput").ap()
    wqT = nc.dram_tensor("wqT", [H, QF], bf16, kind="ExternalInput").ap()
    wkT = nc.dram_tensor("wkT", [H, KF], bf16, kind="ExternalInput").ap()
    wvT = nc.dram_tensor("wvT", [H, KF], bf16, kind="ExternalInput").ap()
    woT = nc.dram_tensor("woT", [QF, H], bf16, kind="ExternalInput").ap()
    mb = nc.dram_tensor("mb", [P, KB], f32, kind="ExternalInput").ap()
    outT = nc.dram_tensor("outT", [H, S], bf16, kind="ExternalOutput").ap()

    xT_r = xT.rearrange("(t p) s -> p t s", p=P)
    xkT_r = xkT.rearrange("(t p) s -> p t s", p=P)
    wqT_r = wqT.rearrange("(t p) f -> p t f", p=P)
    wkT_r = wkT.rearrange("(t p) f -> p t f", p=P)
    wvT_r = wvT.rearrange("(t p) f -> p t f", p=P)
    woT_r = woT.rearrange("(t p) f -> p t f", p=P)

    with tile.TileContext(nc) as tc, ExitStack() as es:
        ec = es.enter_context
        ec(nc.allow_low_precision(reason="bf16 matmuls, fp32 PSUM accum"))
        const_pool = ec(tc.tile_pool(name="const", bufs=1))
        wq_pool = ec(tc.tile_pool(name="wq", bufs=1))
        wo_pool = ec(tc.tile_pool(name="wo", bufs=1))
        wkv_pool = ec(tc.tile_pool(name="wkv", bufs=1))
        xt_pool = ec(tc.tile_pool(name="xt", bufs=2))
        xk_pool = ec(tc.tile_pool(name="xk", bufs=3))
        qt_pool = ec(tc.tile_pool(name="qt", bufs=1))
        kt_pool = ec(tc.tile_pool(name="kt", bufs=1))
        v_pool = ec(tc.tile_pool(name="vs", bufs=1))
        at_pool = ec(tc.tile_pool(name="at", bufs=2))
        e_pool = ec(tc.tile_pool(name="e", bufs=4))
        rc_pool = ec(tc.tile_pool(name="rc", bufs=2))
        rb_pool = ec(tc.tile_pool(name="rb", bufs=2))
        out_pool = ec(tc.tile_pool(name="outs", bufs=3))
        psa_pool = ec(tc.tile_pool(name="psa", bufs=2, space="PSUM"))
        po_pool = ec(tc.tile_pool(name="po", bufs=2, space="PSUM"))
        pp_pool = ec(tc.tile_pool(name="pp", bufs=2, space="PSUM"))

        # ---- constants ----
        mb_sb = const_pool.tile([P, KB], f32, tag="mb")
        nc.sync.dma_start(out=mb_sb, in_=mb)
        ones_sb = const_pool.tile([1, D], bf16, tag="ones")
        nc.vector.memset(ones_sb, 1.0)

        # ---- k/v weights + gathered-key x, split per-ht so the PE can
        # start on ht=0 almost immediately; wq interleaves per-mt between
        # xk chunks (xk bufs=3 avoids DMA head-of-line blocking) ----
        wk_sb = wkv_pool.tile([P, NHT, KF], bf16, tag="wk")
        wv_sb = wkv_pool.tile([P, NHT, KF], bf16, tag="wv")
        nc.sync.dma_start(out=wk_sb, in_=wkT_r)
        nc.sync.dma_start(out=wv_sb, in_=wvT_r)

        wq_sb = wq_pool.tile([P, NHT, QF], bf16, tag="wq")
        nkc = (KPAD + SQA - 1) // SQA
        xk_tiles = []
        for c in range(nkc):
            k0 = c * SQA
            kw = min(SQA, KPAD - k0)
            xk = xk_pool.tile([P, NHT, SQA], bf16, tag="xk", name=f"xk{c}")
            if c == 0:
                # split the first chunk so the PE starts after ~1/4 of it
                for hq in range(4):
                    nc.sync.dma_start(
                        out=xk[:, 4 * hq:4 * hq + 4, 0:kw],
                        in_=xkT_r[:, 4 * hq:4 * hq + 4, k0:k0 + kw],
                    )
            else:
                nc.sync.dma_start(out=xk[:, :, 0:kw], in_=xkT_r[:, :, k0:k0 + kw])
            xk_tiles.append(xk)
        # wq in halves: q-proj half 0 (mts 0,1) can start sooner
        nc.sync.dma_start(out=wq_sb[:, :, 0:2 * P], in_=wqT_r[:, :, 0:2 * P])

        # kT stored twice: kta = [g0; g1] on partitions [0:64; 64:128],
        # ktb = [g1; g0] — so any (group, q-parity) pair can be read at
        # the base partition (matmul requires lhsT base == rhs base).
        kta_sb = kt_pool.tile([P, KPAD], bf16, tag="kta")
        ktb_sb = kt_pool.tile([P, KPAD], bf16, tag="ktb")
        # per-group v tiles: [v (64) | ones] per key block (XBAR transpose
        # output must land at an aligned free offset, so each group gets
        # its own tile with the transpose writing at offset 0)
        # kb-block stride padded to 128 so every transpose destination
        # offset is XBAR-aligned; cols 65..127 are unused padding
        v_g = [
            v_pool.tile([P, KB, P], bf16, tag=f"v{g}", name=f"v{g}")
            for g in range(2)
        ]
        onescol_sb = const_pool.tile([P, KB], bf16, tag="onescol")
        nc.vector.memset(onescol_sb, 1.0)
        nc.vector.tensor_copy(v_g[0][:, :, 64], onescol_sb)
        nc.vector.tensor_copy(v_g[1][:, :, 64], onescol_sb)

        # ---- phase A1: k/v projections over kept keys ----
        for c in range(nkc):
            k0 = c * SQA
            kw = min(SQA, KPAD - k0)
            xk = xk_tiles[c]
            ps = pp_pool.tile([P, SQA], f32, tag="pp", name="ps_k")
            for ht in range(NHT):
                nc.tensor.matmul(
                    ps[:, 0:kw],
                    lhsT=wk_sb[:, ht, :],
                    rhs=xk[:, ht, 0:kw],
                    start=(ht == 0),
                    stop=(ht == NHT - 1),
                )
            nc.scalar.copy(kta_sb[:, k0:k0 + kw], ps[:, 0:kw])
            nc.vector.tensor_copy(ktb_sb[0:64, k0:k0 + kw], ps[64:128, 0:kw])
            nc.vector.tensor_copy(ktb_sb[64:128, k0:k0 + kw], ps[0:64, 0:kw])
            # v (key-major): out[k, vf]
            for st in range(kw // P):
                kb = (k0 + st * P) // P
                psv = pp_pool.tile([P, KF], f32, tag="pp", name="ps_v")
                for ht in range(NHT):
                    nc.tensor.matmul(
                        psv,
                        lhsT=xk[:, ht, st * P:(st + 1) * P],
                        rhs=wv_sb[:, ht, :],
                        start=(ht == 0),
                        stop=(ht == NHT - 1),
                    )
                nc.scalar.copy(v_g[0][:, kb, 0:64], psv[:, 0:64])
                nc.scalar.copy(v_g[1][:, kb, 0:64], psv[:, 64:128])

        # wo streams in after xt0 (overlaps q-proj + attention start)
        wo_sb = wo_pool.tile([P, NMT, H], bf16, tag="wo")

        # ---- q projection, one q-tile at a time (xt DMA + wo after
        # tile 0's, interleaved into attention of the previous tile) ----
        # qT stored with col = (qtile, mt, q) so a head-pair's q tile is
        # one contiguous 2*SQB range (matmul moving AP must be 1D).
        qt_sb = qt_pool.tile([P, S * NMT], bf16, tag="qt")
        xt_tiles = {}

        def xt_dma(qt):
            s0 = qt * SQB
            xt = xt_pool.tile([P, NHT, SQB], bf16, tag="xt", name=f"xt{qt}")
            nc.sync.dma_start(out=xt, in_=xT_r[:, :, s0:s0 + SQB])
            xt_tiles[qt] = xt

        def qproj_half(qt, half):
            xt = xt_tiles[qt]
            for mt in range(2 * half, 2 * half + 2):
                ps = pp_pool.tile([P, SQB], f32, tag="pp", name="ps_q")
                for ht in range(NHT):
                    nc.tensor.matmul(
                        ps,
                        lhsT=wq_sb[:, ht, mt * P:(mt + 1) * P],
                        rhs=xt[:, ht, :],
                        start=(ht == 0),
                        stop=(ht == NHT - 1),
                    )
                c0 = (qt * NMT + mt) * SQB
                nc.scalar.copy(qt_sb[:, c0:c0 + SQB], ps)

        xt_dma(0)
        nc.sync.dma_start(
            out=wq_sb[:, :, 2 * P:4 * P], in_=wqT_r[:, :, 2 * P:4 * P]
        )
        nc.sync.dma_start(out=wo_sb, in_=woT_r)
        xt_dma(1)
        qproj_half(0, 0)

        # ---- phase B/C: attention per q tile; q-proj of tile qt+1 and
        # out-projection of tile qt-1 fill the pair boundaries ----
        # Heads sharing a (group, q-parity) merge into one 2*SQB-wide
        # stream: pair hp covers heads at mt0=2*(hp//2)+{0,1}, partition
        # base r0=64*(hp%2), group g=hp//2.
        at_tiles = {}

        def outproj_8(qt, part):
            q0 = qt * SQB
            for mt in range(part * 8, part * 8 + 8):
                ps = pp_pool.tile([P, SQB], f32, tag="pp", name="ps_o")
                for kb4 in range(NMT):
                    nc.tensor.matmul(
                        ps,
                        lhsT=wo_sb[:, kb4, mt * P:(mt + 1) * P],
                        rhs=at_tiles[(qt, kb4)][:, :],
                        start=(kb4 == 0),
                        stop=(kb4 == NMT - 1),
                    )
                ot = out_pool.tile([P, SQB], bf16, tag="ot")
                nc.vector.tensor_copy(ot, ps)
                nc.sync.dma_start(
                    out=outT[mt * P:(mt + 1) * P, q0:q0 + SQB], in_=ot
                )

        for qt in range(NQT):
            for mt in range(NMT):
                at_tiles[(qt, mt)] = at_pool.tile(
                    [P, SQB], bf16, tag=f"at{mt}", name=f"at_{qt}_{mt}"
                )
            for hp in range(4):
                g = hp // 2
                par = hp % 2
                r0 = par * D
                mt0 = 2 * g
                kt_src = kta_sb if g == par else ktb_sb
                po = [
                    po_pool.tile([65, SQB], f32, tag="po", name=f"po{j}")
                    for j in range(2)
                ]
                for kb in range(KB):
                    # matmul PSUM writes are bank-scoped (<=512 f32
                    # cols): two matmuls fill the wide tile's halves.
                    ps = psa_pool.tile([P, 2 * SQB], f32, tag="ps")
                    for j in range(2):
                        nc.tensor.matmul(
                            ps[:, j * SQB:(j + 1) * SQB],
                            lhsT=kt_src[r0:r0 + D, kb * P:(kb + 1) * P],
                            rhs=qt_sb[r0:r0 + D,
                                      (qt * NMT + mt0 + j) * SQB:
                                      (qt * NMT + mt0 + j + 1) * SQB],
                            start=True,
                            stop=True,
                        )
                    e = e_pool.tile([P, 2 * SQB], bf16, tag="e")
                    nc.scalar.activation(
                        e, ps, Exp,
                        bias=mb_sb[:, kb:kb + 1], scale=float(SCALE),
                    )
                    for j in range(2):
                        nc.tensor.matmul(
                            po[j],
                            lhsT=v_g[g][:, kb, 0:65],
                            rhs=e[:, j * SQB:(j + 1) * SQB],
                            start=(kb == 0),
                            stop=(kb == KB - 1),
                        )
                # normalize: rows 0..63 are numerator^T, row 64 denominator
                # (copy denom to partition 0 first: custom-DVE recip
                # mis-reads nonzero partition bases)
                dn = rc_pool.tile([1, 2 * SQB], f32, tag="dn")
                for j in range(2):
                    nc.vector.tensor_copy(
                        dn[:, j * SQB:(j + 1) * SQB], po[j][64:65, :]
                    )
                rc = rc_pool.tile([1, 2 * SQB], f32, tag="rc")
                nc.vector.reciprocal_approx_fast(rc, dn)
                rb = rb_pool.tile([D, 2 * SQB], f32, tag="rb")
                nc.gpsimd.partition_broadcast(rb, rc)
                for j in range(2):
                    nc.vector.tensor_mul(
                        at_tiles[(qt, mt0 + j)][r0:r0 + D, :],
                        po[j][0:64, :],
                        rb[:, j * SQB:(j + 1) * SQB],
                    )
                # fill the pair boundary with independent PE work
                if qt == 0:
                    if hp == 0:
                        qproj_half(0, 1)
                    elif hp == 1:
                        qproj_half(1, 0)
                    elif hp == 2:
                        qproj_half(1, 1)
                    else:
                        xt_dma(2)
                elif qt < NQT - 1:
                    if hp <= 1:
                        qproj_half(qt + 1, hp)
                    else:
                        if hp == 2 and qt + 2 < NQT:
                            xt_dma(qt + 2)
                        outproj_8(qt - 1, hp - 2)
                else:
                    if hp >= 2:
                        outproj_8(qt - 1, hp - 2)
        for part in range(2):
            outproj_8(NQT - 1, part)
    nc.compile()
    return nc


_NC_CACHE = {}


def _get_nc(KB):
    if KB not in _NC_CACHE:
        _NC_CACHE[KB] = _build_bass(KB)
    return _NC_CACHE[KB]


def _make_in_maps(inputs):
    import ml_dtypes

    bf = ml_dtypes.bfloat16
    x = np.asarray(inputs["x"], dtype=np.float32)
    mask = np.asarray(inputs["mask"])
    Wq = np.asarray(inputs["Wq"], dtype=np.float32)
    Wk = np.asarray(inputs["Wk"], dtype=np.float32)
    Wv = np.asarray(inputs["Wv"], dtype=np.float32)
    Wo = np.asarray(inputs["Wo"], dtype=np.float32)

    # gather kept (unmasked) key positions per batch; pad to common KPAD
    idxs = [np.nonzero(mask[b, 0, 0, 0, :] != 0)[0] for b in range(B)]
    kept_max = max(1, max(len(i) for i in idxs))
    KB = (kept_max + P - 1) // P
    KPAD = KB * P

    xTs, xkTs, mbs = [], [], []
    for b in range(B):
        xb = x[b].astype(bf)
        xTs.append(np.ascontiguousarray(xb.T))
        xk = np.zeros((KPAD, H), dtype=bf)
        xk[: len(idxs[b])] = xb[idxs[b]]
        xkTs.append(np.ascontiguousarray(xk.T))
        bias = np.full(KPAD, np.float32(MASK_NEG), dtype=np.float32)
        bias[: len(idxs[b])] = 0.0
        mbs.append(np.ascontiguousarray(bias.reshape(KB, P).T))

    in_maps = []
    for c in range(NCORES):
        b, gp = divmod(c, 4)
        qs = slice(gp * QF, (gp + 1) * QF)
        ks = slice(gp * KF, (gp + 1) * KF)
        in_maps.append({
            "xT": xTs[b],
            "xkT": xkTs[b],
            "wqT": np.ascontiguousarray(Wq[qs, :].T.astype(bf)),
            "wkT": np.ascontiguousarray(Wk[ks, :].T.astype(bf)),
            "wvT": np.ascontiguousarray(Wv[ks, :].T.astype(bf)),
            "woT": np.ascontiguousarray(Wo[:, qs].T.astype(bf)),
            "mb": mbs[b],
        })
    return in_maps, KB


def kernel(**inputs):
    from concourse.bass_utils import run_bass_kernel_spmd

    in_maps, KB = _make_in_maps(inputs)
    nc = _get_nc(KB)
    res = run_bass_kernel_spmd(nc, in_maps, core_ids=list(range(NCORES)))
    outs = [np.asarray(r["outT"], dtype=np.float32) for r in res.results]
    out = np.empty((B, S, H), dtype=np.float32)
    for b in range(B):
        acc = outs[4 * b] + outs[4 * b + 1] + outs[4 * b + 2] + outs[4 * b + 3]
        out[b] = acc.T
    return out
